# revision 19
# baseline (speedup 1.0000x reference)
"""Trainium2 Bass kernel for nn_MixtureOfExperts_57045755625494.

Expert-parallel across 8 NeuronCores: core c owns expert e=c. The host
computes top-2 routing (fp32) only to build per-expert token gather lists;
all reference math runs on device:
  - fp32 router (PE matmul + exp/tanh on ACT) -> top-2 combine weights
  - dense SwiGLU FFN on the gathered token set (fp16 matmuls, fp32 PSUM)
  - per-token scatter back to a token-major partial buffer (indirect DMA)
  - ReduceScatter across the 8 cores combines expert contributions
  - aux-loss partial sums (counts / prob sums / exp-sums / cap sums)
Host gathers shards and finishes the four scalar losses from the
device-computed partials.

kernel(**inputs) takes the full unsharded inputs and returns
(out[2,2048,768] fp32, load_balance_loss, router_z_loss, diversity_loss,
capacity_loss) matching reference().
"""
import numpy as np

import concourse.bass as bass
import concourse.bacc as bacc
import concourse.mybir as mybir
import concourse.tile as tile
import bass_rust
from concourse.bass_utils import run_bass_kernel_spmd

AF = bass_rust.ActivationFunctionType
ALU = mybir.AluOpType
DT = mybir.dt

B, S, H, I, E, K = 2, 2048, 768, 3072, 8, 2
T = B * S
N_CORES = 8
NH = H // 128
NI = I // 128
PAD_IDX = 1 << 20

try:
    AXIS_X = mybir.AxisListType.X
except AttributeError:
    AXIS_X = bass_rust.AxisListType.X


def _build(C, w_dtype=DT.float16):
    NB = C // 128
    # even-ish token tiles (multiples of 128, each <=512) to avoid a tiny
    # LDW-bound tail tile
    n_t = -(-C // 512)
    per = [C // n_t // 128] * n_t
    for j in range((C - sum(per) * 128) // 128):
        per[j] += 1
    tiles = []
    off = 0
    for blocks in per:
        tiles.append((off, blocks * 128))
        off += blocks * 128

    nc = bacc.Bacc("TRN2", target_bir_lowering=False, debug=False, num_devices=N_CORES)

    xg32 = nc.dram_tensor("xg32", [H, C], DT.float32, kind="ExternalInput").ap()
    xg16 = nc.dram_tensor("xg16", [H, C], w_dtype, kind="ExternalInput").ap()
    rcT = nc.dram_tensor("rcT", [H, 40], DT.float32, kind="ExternalInput").ap()
    capb = nc.dram_tensor("capb128", [128, 1], DT.float32, kind="ExternalInput").ap()
    wsel = nc.dram_tensor("wsel", [128, E], DT.float32, kind="ExternalInput").ap()
    idxc = nc.dram_tensor("idxcol", [128, NB], DT.int32, kind="ExternalInput").ap()
    vmc = nc.dram_tensor("vmcol", [128, NB], DT.float32, kind="ExternalInput").ap()
    gwT = nc.dram_tensor("gwT", [H, I], w_dtype, kind="ExternalInput").ap()
    uwT = nc.dram_tensor("uwT", [H, I], w_dtype, kind="ExternalInput").ap()
    dwT = nc.dram_tensor("dwT", [I, H], w_dtype, kind="ExternalInput").ap()

    out_shard = nc.dram_tensor("out_shard", [T // N_CORES, H], w_dtype, kind="ExternalOutput").ap()
    aux_o = nc.dram_tensor("aux", [1, 24], DT.float32, kind="ExternalOutput").ap()
    spz_o = nc.dram_tensor("spz", [128, NB], DT.float32, kind="ExternalOutput").ap()

    with tile.TileContext(nc) as tc:
        with (
            tc.tile_pool(name="wres", bufs=1) as wres,
            tc.tile_pool(name="rt", bufs=8) as rt,
            tc.tile_pool(name="hidp", bufs=NI + 6) as hidp,
            tc.tile_pool(name="epi", bufs=3) as epi,
            tc.tile_pool(name="ps", bufs=2, space="PSUM") as ps,
            tc.tile_pool(name="dram", bufs=1, space="DRAM") as dram,
        ):
            NQ = 4
            IQ = I // NQ
            gw_sb, uw_sb, dw_sb, rc_sb, x32_sb, x16_sb = [], [], [], [], [], []
            capb_sb = wres.tile([128, 1], DT.float32, tag="capb")
            nc.sync.dma_start(out=capb_sb[:], in_=capb[:])
            wsel_sb = wres.tile([128, E], DT.float32, tag="wsel")
            nc.sync.dma_start(out=wsel_sb[:], in_=wsel[:])
            idx_sb = wres.tile([128, NB], DT.int32, tag="idx")
            nc.sync.dma_start(out=idx_sb[:], in_=idxc[:])
            vm_sb = wres.tile([128, NB], DT.float32, tag="vm")
            nc.sync.dma_start(out=vm_sb[:], in_=vmc[:])
            for h in range(NH):
                t_ = wres.tile([128, 40], DT.float32, tag=f"rc{h}")
                nc.sync.dma_start(out=t_[:], in_=rcT[h * 128:(h + 1) * 128, :])
                rc_sb.append(t_)
            # x fp32 split by token tile: router blocks depend only on their chunk
            for h in range(NH):
                row = []
                for ti, (c0, sz) in enumerate(tiles):
                    t_ = wres.tile([128, sz], DT.float32, tag=f"x32{h}_{ti}", name=f"x32_{h}_{ti}")
                    nc.sync.dma_start(out=t_[:], in_=xg32[h * 128:(h + 1) * 128, c0:c0 + sz])
                    row.append(t_)
                x32_sb.append(row)
            # gate/up weights split into NQ column groups: GEMM1 i-chunk j needs only group j*NQ//NI
            for h in range(NH):
                gw_sb.append([None] * NQ)
                uw_sb.append([None] * NQ)
            for q in range(NQ):
                for h in range(NH):
                    t_ = wres.tile([128, IQ], w_dtype, tag=f"gw{h}_{q}", name=f"gw_{h}_{q}")
                    nc.sync.dma_start(out=t_[:], in_=gwT[h * 128:(h + 1) * 128, q * IQ:(q + 1) * IQ])
                    gw_sb[h][q] = t_
                for h in range(NH):
                    t_ = wres.tile([128, IQ], w_dtype, tag=f"uw{h}_{q}", name=f"uw_{h}_{q}")
                    nc.sync.dma_start(out=t_[:], in_=uwT[h * 128:(h + 1) * 128, q * IQ:(q + 1) * IQ])
                    uw_sb[h][q] = t_
                if q == 0:
                    for h in range(NH):
                        row = []
                        for ti, (c0, sz) in enumerate(tiles):
                            t_ = wres.tile([128, sz], w_dtype, tag=f"x16{h}_{ti}", name=f"x16_{h}_{ti}")
                            nc.sync.dma_start(out=t_[:], in_=xg16[h * 128:(h + 1) * 128, c0:c0 + sz])
                            row.append(t_)
                        x16_sb.append(row)
            for i in range(NI):
                t_ = wres.tile([128, H], w_dtype, tag=f"dw{i}")
                nc.sync.dma_start(out=t_[:], in_=dwT[i * 128:(i + 1) * 128, :])
                dw_sb.append(t_)

            ones_sb = wres.tile([128, 1], DT.float32, tag="ones")
            nc.vector.memset(ones_sb[:], 1.0)
            zero_sb = wres.tile([128, H], w_dtype, tag="zero")
            nc.vector.memset(zero_sb[:], 0.0)

            wcol_all = wres.tile([128, NB], DT.float32, tag="wcol")
            cnt_acc = wres.tile([128, E], DT.float32, tag="cnta")
            pbar_acc = wres.tile([128, E], DT.float32, tag="pbara")
            spz_sb = wres.tile([128, NB], DT.float32, tag="spz")
            cap_acc = wres.tile([128, 1], DT.float32, tag="capacc")
            nc.vector.memset(cnt_acc[:], 0.0)
            nc.vector.memset(pbar_acc[:], 0.0)
            nc.vector.memset(cap_acc[:], 0.0)

            partial = dram.tile([T, H], w_dtype)
            rs_out = dram.tile([T // N_CORES, H], w_dtype)

            for r in range(T // 128):
                nc.sync.dma_start(out=partial[r * 128:(r + 1) * 128, :], in_=zero_sb[:])

            # ---- router on all token blocks (token-major: tokens on partitions) ----
            for b in range(NB):
                ti = next(j for j, (c0_, sz_) in enumerate(tiles) if c0_ <= b * 128 < c0_ + sz_)
                loc = b * 128 - tiles[ti][0]
                psr = ps.tile([128, 40], DT.float32, tag="psg")
                for h in range(NH):
                    nc.tensor.matmul(psr[:], x32_sb[h][ti][:, loc:loc + 128], rc_sb[h][:],
                                     start=(h == 0), stop=(h == NH - 1))
                L = rt.tile([128, 40], DT.float32, tag="r")
                nc.scalar.activation(L[:], psr[:], AF.Copy)
                E8 = rt.tile([128, E], DT.float32, tag="r")
                nc.scalar.activation(E8[:], L[:, 0:E], AF.Exp)
                Ssum = rt.tile([128, 1], DT.float32, tag="rs")
                nc.vector.tensor_reduce(Ssum[:], E8[:], AXIS_X, ALU.add)
                R = rt.tile([128, 1], DT.float32, tag="rs")
                nc.vector.reciprocal(R[:], Ssum[:])
                P = rt.tile([128, E], DT.float32, tag="r")
                nc.vector.tensor_scalar_mul(P[:], E8[:], R[:])
                M1 = rt.tile([128, 1], DT.float32, tag="rs")
                nc.vector.tensor_reduce(M1[:], P[:], AXIS_X, ALU.max)
                msk1 = rt.tile([128, E], DT.float32, tag="r")
                nc.vector.tensor_scalar(msk1[:], P[:], M1[:], None, ALU.is_ge)
                Pm = rt.tile([128, E], DT.float32, tag="r")
                nc.vector.scalar_tensor_tensor(Pm[:], msk1[:], -1e30, P[:], ALU.mult, ALU.add)
                M2 = rt.tile([128, 1], DT.float32, tag="rs")
                nc.vector.tensor_reduce(M2[:], Pm[:], AXIS_X, ALU.max)
                msk2 = rt.tile([128, E], DT.float32, tag="r")
                nc.vector.tensor_scalar(msk2[:], P[:], M2[:], None, ALU.is_equal)
                e1 = rt.tile([128, 1], DT.float32, tag="rs")
                nc.scalar.activation(e1[:], M1[:], AF.Exp)
                e2 = rt.tile([128, 1], DT.float32, tag="rs")
                nc.scalar.activation(e2[:], M2[:], AF.Exp)
                den = rt.tile([128, 1], DT.float32, tag="rs")
                nc.vector.tensor_tensor(den[:], e1[:], e2[:], ALU.add)
                rd = rt.tile([128, 1], DT.float32, tag="rs")
                nc.vector.reciprocal(rd[:], den[:])
                # dyn_cap via tanh (keeps ACT on one table set):
                # sigmoid(z) = 0.5*tanh(0.5 z) + 0.5 ; capb128 is pre-scaled by 0.5
                dc = rt.tile([128, 1], DT.float32, tag="rs")
                nc.scalar.activation(dc[:], L[:, 32:33], AF.Tanh, bias=capb_sb[:], scale=0.5)
                nc.vector.tensor_scalar(dc[:], dc[:], 0.5, 0.5, ALU.mult, ALU.add)
                s1 = rt.tile([128, 1], DT.float32, tag="rs")
                nc.vector.tensor_tensor(s1[:], e1[:], rd[:], ALU.mult)
                nc.vector.tensor_tensor(s1[:], s1[:], dc[:], ALU.mult)
                s2 = rt.tile([128, 1], DT.float32, tag="rs")
                nc.vector.tensor_tensor(s2[:], e2[:], rd[:], ALU.mult)
                nc.vector.tensor_tensor(s2[:], s2[:], dc[:], ALU.mult)
                w8 = rt.tile([128, E], DT.float32, tag="r")
                nc.vector.tensor_scalar_mul(w8[:], msk1[:], s1[:])
                tmp = rt.tile([128, E], DT.float32, tag="r")
                nc.vector.tensor_scalar_mul(tmp[:], msk2[:], s2[:])
                nc.vector.tensor_tensor(w8[:], w8[:], tmp[:], ALU.add)
                nc.vector.tensor_tensor(w8[:], w8[:], wsel_sb[:], ALU.mult)
                nc.vector.tensor_reduce(wcol_all[:, b:b + 1], w8[:], AXIS_X, ALU.add)

                vmb = vm_sb[:, b:b + 1]
                m12 = rt.tile([128, E], DT.float32, tag="r")
                nc.vector.tensor_tensor(m12[:], msk1[:], msk2[:], ALU.add)
                nc.vector.tensor_scalar_mul(m12[:], m12[:], vmb)
                nc.vector.tensor_tensor(cnt_acc[:], cnt_acc[:], m12[:], ALU.add)
                Pv = rt.tile([128, E], DT.float32, tag="r")
                nc.vector.tensor_scalar_mul(Pv[:], P[:], vmb)
                nc.vector.tensor_tensor(pbar_acc[:], pbar_acc[:], Pv[:], ALU.add)
                EP = rt.tile([128, E], DT.float32, tag="r")
                nc.scalar.activation(EP[:], P[:], AF.Exp)
                nc.vector.tensor_reduce(spz_sb[:, b:b + 1], EP[:], AXIS_X, ALU.add)
                dcv = rt.tile([128, 1], DT.float32, tag="rs")
                nc.vector.tensor_tensor(dcv[:], dc[:], vmb, ALU.mult)
                nc.vector.tensor_tensor(cap_acc[:], cap_acc[:], dcv[:], ALU.add)

            def emit_aux():
                # ---- aux partial sums -> one row via ones-matmul ----
                psx = ps.tile([1, 24], DT.float32, tag="psg")
                aux_cat = wres.tile([128, 24], DT.float32, tag="auxcat")
                nc.vector.memset(aux_cat[:], 0.0)
                nc.vector.tensor_copy(aux_cat[:, 0:E], cnt_acc[:])
                nc.vector.tensor_copy(aux_cat[:, E:2 * E], pbar_acc[:])
                nc.vector.tensor_copy(aux_cat[:, 2 * E:2 * E + 1], cap_acc[:])
                nc.tensor.matmul(psx[:], ones_sb[:], aux_cat[:], start=True, stop=True)
                aux_sb = wres.tile([1, 24], DT.float32, tag="auxsb")
                nc.vector.tensor_copy(aux_sb[:], psx[:])
                nc.sync.dma_start(out=aux_o[:], in_=aux_sb[:])
                nc.sync.dma_start(out=spz_o[:], in_=spz_sb[:])

            # ---- SwiGLU FFN on the gathered token set ----
            for ti, (c0, sz) in enumerate(tiles):
                hid = []
                for i in range(NI):
                    q, iloc = i * NQ // NI, (i % (NI // NQ)) * 128
                    psg = ps.tile([128, 512], DT.float32, tag="psg")
                    for h in range(NH):
                        nc.tensor.matmul(psg[:, 0:sz], gw_sb[h][q][:, iloc:iloc + 128],
                                         x16_sb[h][ti][:],
                                         start=(h == 0), stop=(h == NH - 1))
                    psu = ps.tile([128, 512], DT.float32, tag="psu")
                    for h in range(NH):
                        nc.tensor.matmul(psu[:, 0:sz], uw_sb[h][q][:, iloc:iloc + 128],
                                         x16_sb[h][ti][:],
                                         start=(h == 0), stop=(h == NH - 1))
                    # silu(g)*u with ACT on one table set: sigmoid(g)=0.5*tanh(g/2)+0.5
                    # so silu(g)*u = 0.5*g*((tanh(g/2)+1)*u)
                    th = epi.tile([128, 512], DT.float32, tag="th")
                    nc.scalar.activation(th[:, 0:sz], psg[:, 0:sz], AF.Tanh, scale=0.5)
                    sg = epi.tile([128, 512], DT.float32, tag="sg")
                    nc.vector.scalar_tensor_tensor(sg[:, 0:sz], th[:, 0:sz], 1.0, psu[:, 0:sz], ALU.add, ALU.mult)
                    ht = hidp.tile([128, 512], w_dtype, tag="hid")
                    nc.vector.scalar_tensor_tensor(ht[:, 0:sz], sg[:, 0:sz], 0.5, psg[:, 0:sz], ALU.mult, ALU.mult)
                    hid.append(ht)
                for tb in range(sz // 128):
                    b = (c0 + tb * 128) // 128
                    t0 = tb * 128
                    psyA = ps.tile([128, 384], DT.float32, tag="psyA")
                    psyB = ps.tile([128, 384], DT.float32, tag="psyB")
                    for i in range(NI):
                        nc.tensor.matmul(psyA[:], hid[i][:, t0:t0 + 128], dw_sb[i][:, 0:384],
                                         start=(i == 0), stop=(i == NI - 1))
                    for i in range(NI):
                        nc.tensor.matmul(psyB[:], hid[i][:, t0:t0 + 128], dw_sb[i][:, 384:768],
                                         start=(i == 0), stop=(i == NI - 1))
                    y_sb = epi.tile([128, H], w_dtype, tag="ysb")
                    nc.vector.tensor_scalar_mul(y_sb[:, 0:384], psyA[:], wcol_all[:, b:b + 1])
                    nc.vector.tensor_scalar_mul(y_sb[:, 384:768], psyB[:], wcol_all[:, b:b + 1])
                    nc.gpsimd.indirect_dma_start(
                        out=partial[:],
                        out_offset=bass.IndirectOffsetOnAxis(ap=idx_sb[:, b:b + 1], axis=0),
                        in_=y_sb[:],
                        in_offset=None,
                        bounds_check=T - 1,
                        oob_is_err=False,
                    )
                if ti == 0:
                    emit_aux()

            # ---- combine expert contributions across the 8 cores ----
            nc.gpsimd.collective_compute(
                "ReduceScatter",
                ALU.add,
                replica_groups=[list(range(N_CORES))],
                ins=[partial.opt()],
                outs=[rs_out.opt()],
            )
            nc.sync.dma_start(out=out_shard[:], in_=rs_out[:])

    nc.compile()
    return nc


_NC_CACHE = {}


def _get_nc(C):
    if C not in _NC_CACHE:
        _NC_CACHE[C] = _build(C)
    return _NC_CACHE[C]


def kernel(x, router_w, cap_w, cap_b, gate_w, up_w, down_w):
    x = np.asarray(x, np.float32)
    router_w = np.asarray(router_w, np.float32)
    cap_w = np.asarray(cap_w, np.float32)
    cap_b = np.asarray(cap_b, np.float32)
    gate_w = np.asarray(gate_w, np.float32)
    up_w = np.asarray(up_w, np.float32)
    down_w = np.asarray(down_w, np.float32)
    w_np = np.float16

    xf = x.reshape(T, H)
    xT = np.ascontiguousarray(xf.T)                      # [H, T]
    rc = np.zeros((40, H), np.float32)
    rc[:E] = router_w
    rc[32] = cap_w[0]
    rcT = np.ascontiguousarray(rc.T)                     # [H, 40]
    capb128 = np.broadcast_to(0.5 * cap_b.reshape(1, 1), (128, 1)).astype(np.float32).copy()

    # host top-2 routing (only to build the expert-parallel gather lists)
    logits = xf @ router_w.T                             # [T, E]
    i1 = np.argmax(logits, axis=1)
    l2 = logits.copy()
    l2[np.arange(T), i1] = -np.inf
    i2 = np.argmax(l2, axis=1)

    tok_lists = [np.where((i1 == c) | (i2 == c))[0] for c in range(N_CORES)]
    max_cnt = max(len(t) for t in tok_lists)
    C = max(1152, -(-max_cnt // 128) * 128)              # capacity (block-multiple)
    NB = C // 128
    nc = _get_nc(C)

    in_maps = []
    vmcols = []
    for c in range(N_CORES):
        toks = tok_lists[c]
        n = len(toks)
        idx_arr = np.full(C, PAD_IDX, np.int64)
        idx_arr[:n] = toks
        vm_arr = np.zeros(C, np.float32)
        vm_arr[:n] = 1.0
        gcols = np.where(idx_arr < T, idx_arr, 0)
        xg32 = np.ascontiguousarray(xT[:, gcols])
        wsel = np.zeros((128, E), np.float32)
        wsel[:, c] = 1.0
        vmcol = np.ascontiguousarray(vm_arr.reshape(NB, 128).T)
        vmcols.append(vmcol)
        in_maps.append({
            "xg32": xg32,
            "xg16": xg32.astype(w_np),
            "rcT": rcT,
            "capb128": capb128,
            "wsel": wsel,
            "idxcol": np.ascontiguousarray(idx_arr.reshape(NB, 128).T.astype(np.int32)),
            "vmcol": vmcol,
            "gwT": np.ascontiguousarray(gate_w[c].T).astype(w_np),
            "uwT": np.ascontiguousarray(up_w[c].T).astype(w_np),
            "dwT": np.ascontiguousarray(down_w[c].T).astype(w_np),
        })

    res = run_bass_kernel_spmd(nc, in_maps, list(range(N_CORES)))
    results = res.results

    y = np.concatenate([results[c]["out_shard"] for c in range(N_CORES)], axis=0)
    out = y.astype(np.float32).reshape(B, S, H)

    aux = np.stack([results[c]["aux"][0] for c in range(N_CORES)])       # [8, 24]
    counts = np.array([aux[c, c] for c in range(N_CORES)], np.float64)
    pbar = aux[:, E:2 * E].sum(axis=0) / 2.0 / T
    capm = aux[:, 2 * E].sum() / 2.0 / T
    zsum = 0.0
    for c in range(N_CORES):
        spz = results[c]["spz"].astype(np.float64)
        zsum += float((np.log(np.where(spz > 0, spz, 1.0)) * vmcols[c]).sum())
    zsum = zsum / 2.0 / T

    lbl = np.float32(counts.var(ddof=1) / (T * K / E) ** 2)
    rzl = np.float32(zsum)
    dl = np.float32(-(pbar * np.log(pbar + 1e-8)).sum())
    cl = np.float32((capm - 0.6) ** 2)
    return (out, lbl, rzl, dl, cl)


# revision 21
# speedup vs baseline: 1.1426x; 1.1426x over previous
"""Trainium2 Bass kernel for nn_MixtureOfExperts_57045755625494.

Expert-parallel across 8 NeuronCores: core c owns expert e=c. The host
computes top-2 routing (fp32) only to build per-expert token gather lists;
all reference math runs on device:
  - fp32 router (PE matmul + exp/tanh on ACT) -> top-2 combine weights
  - dense SwiGLU FFN on the gathered token set (fp16 matmuls, fp32 PSUM)
  - per-token scatter back to a token-major partial buffer (indirect DMA)
  - ReduceScatter across the 8 cores combines expert contributions
  - aux-loss partial sums (counts / prob sums / exp-sums / cap sums)
Host gathers shards and finishes the four scalar losses from the
device-computed partials.

kernel(**inputs) takes the full unsharded inputs and returns
(out[2,2048,768] fp32, load_balance_loss, router_z_loss, diversity_loss,
capacity_loss) matching reference().
"""
import numpy as np

import concourse.bass as bass
import concourse.bacc as bacc
import concourse.mybir as mybir
import concourse.tile as tile
import bass_rust
from concourse.bass_utils import run_bass_kernel_spmd

AF = bass_rust.ActivationFunctionType
ALU = mybir.AluOpType
DT = mybir.dt

B, S, H, I, E, K = 2, 2048, 768, 3072, 8, 2
T = B * S
N_CORES = 8
NH = H // 128
NI = I // 128
PAD_IDX = 1 << 20

try:
    AXIS_X = mybir.AxisListType.X
except AttributeError:
    AXIS_X = bass_rust.AxisListType.X


def _build(C, RB=None, w_dtype=DT.float16):
    # R: row boundary for the split two-RS variant (blocks 0-4 scatter rows <R,
    # blocks 5.. scatter rows >=R), enabling the first ReduceScatter to overlap
    # the remaining FFN. R=None keeps the single-RS path.
    NB = C // 128
    # even-ish token tiles (multiples of 128, each <=512) to avoid a tiny
    # LDW-bound tail tile
    n_t = -(-C // 512)
    per = [C // n_t // 128] * n_t
    for j in range((C - sum(per) * 128) // 128):
        per[j] += 1
    tiles = []
    off = 0
    for blocks in per:
        tiles.append((off, blocks * 128))
        off += blocks * 128

    nc = bacc.Bacc("TRN2", target_bir_lowering=False, debug=False, num_devices=N_CORES)

    xg32 = nc.dram_tensor("xg32", [H, C], DT.float32, kind="ExternalInput").ap()
    xg16 = nc.dram_tensor("xg16", [H, C], w_dtype, kind="ExternalInput").ap()
    rcT = nc.dram_tensor("rcT", [H, 40], DT.float32, kind="ExternalInput").ap()
    capb = nc.dram_tensor("capb128", [128, 1], DT.float32, kind="ExternalInput").ap()
    wsel = nc.dram_tensor("wsel", [128, E], DT.float32, kind="ExternalInput").ap()
    idxc = nc.dram_tensor("idxcol", [128, NB], DT.int32, kind="ExternalInput").ap()
    vmc = nc.dram_tensor("vmcol", [128, NB], DT.float32, kind="ExternalInput").ap()
    gwT = nc.dram_tensor("gwT", [H, I], w_dtype, kind="ExternalInput").ap()
    uwT = nc.dram_tensor("uwT", [H, I], w_dtype, kind="ExternalInput").ap()
    dwT = nc.dram_tensor("dwT", [I, H], w_dtype, kind="ExternalInput").ap()

    if RB is None:
        out_shard = nc.dram_tensor("out_shard", [T // N_CORES, H], w_dtype, kind="ExternalOutput").ap()
    else:
        out_lo = nc.dram_tensor("out_lo", [RB // N_CORES, H], w_dtype, kind="ExternalOutput").ap()
        out_hi = nc.dram_tensor("out_hi", [(T - RB) // N_CORES, H], w_dtype, kind="ExternalOutput").ap()
    aux_o = nc.dram_tensor("aux", [1, 24], DT.float32, kind="ExternalOutput").ap()
    spz_o = nc.dram_tensor("spz", [128, NB], DT.float32, kind="ExternalOutput").ap()

    with tile.TileContext(nc) as tc:
        with (
            tc.tile_pool(name="wres", bufs=1) as wres,
            tc.tile_pool(name="rt", bufs=8) as rt,
            tc.tile_pool(name="hidp", bufs=NI + 6) as hidp,
            tc.tile_pool(name="epi", bufs=3) as epi,
            tc.tile_pool(name="ps", bufs=2, space="PSUM") as ps,
            tc.tile_pool(name="dram", bufs=1, space="DRAM") as dram,
        ):
            NQ = 4
            IQ = I // NQ
            gw_sb, uw_sb, dw_sb, rc_sb, x32_sb, x16_sb = [], [], [], [], [], []
            capb_sb = wres.tile([128, 1], DT.float32, tag="capb")
            nc.sync.dma_start(out=capb_sb[:], in_=capb[:])
            wsel_sb = wres.tile([128, E], DT.float32, tag="wsel")
            nc.sync.dma_start(out=wsel_sb[:], in_=wsel[:])
            idx_sb = wres.tile([128, NB], DT.int32, tag="idx")
            nc.sync.dma_start(out=idx_sb[:], in_=idxc[:])
            vm_sb = wres.tile([128, NB], DT.float32, tag="vm")
            nc.sync.dma_start(out=vm_sb[:], in_=vmc[:])
            for h in range(NH):
                t_ = wres.tile([128, 40], DT.float32, tag=f"rc{h}")
                nc.sync.dma_start(out=t_[:], in_=rcT[h * 128:(h + 1) * 128, :])
                rc_sb.append(t_)
            # x fp32 split by token tile: router blocks depend only on their chunk
            for h in range(NH):
                row = []
                for ti, (c0, sz) in enumerate(tiles):
                    t_ = wres.tile([128, sz], DT.float32, tag=f"x32{h}_{ti}", name=f"x32_{h}_{ti}")
                    nc.sync.dma_start(out=t_[:], in_=xg32[h * 128:(h + 1) * 128, c0:c0 + sz])
                    row.append(t_)
                x32_sb.append(row)
            # gate/up weights split into NQ column groups: GEMM1 i-chunk j needs only group j*NQ//NI
            for h in range(NH):
                gw_sb.append([None] * NQ)
                uw_sb.append([None] * NQ)
            for q in range(NQ):
                for h in range(NH):
                    t_ = wres.tile([128, IQ], w_dtype, tag=f"gw{h}_{q}", name=f"gw_{h}_{q}")
                    nc.sync.dma_start(out=t_[:], in_=gwT[h * 128:(h + 1) * 128, q * IQ:(q + 1) * IQ])
                    gw_sb[h][q] = t_
                for h in range(NH):
                    t_ = wres.tile([128, IQ], w_dtype, tag=f"uw{h}_{q}", name=f"uw_{h}_{q}")
                    nc.sync.dma_start(out=t_[:], in_=uwT[h * 128:(h + 1) * 128, q * IQ:(q + 1) * IQ])
                    uw_sb[h][q] = t_
                if q == 0:
                    for h in range(NH):
                        row = []
                        for ti, (c0, sz) in enumerate(tiles):
                            t_ = wres.tile([128, sz], w_dtype, tag=f"x16{h}_{ti}", name=f"x16_{h}_{ti}")
                            nc.sync.dma_start(out=t_[:], in_=xg16[h * 128:(h + 1) * 128, c0:c0 + sz])
                            row.append(t_)
                        x16_sb.append(row)
            for i in range(NI):
                t_ = wres.tile([128, H], w_dtype, tag=f"dw{i}")
                nc.sync.dma_start(out=t_[:], in_=dwT[i * 128:(i + 1) * 128, :])
                dw_sb.append(t_)

            ones_sb = wres.tile([128, 1], DT.float32, tag="ones")
            nc.vector.memset(ones_sb[:], 1.0)
            zero_sb = wres.tile([128, H], w_dtype, tag="zero")
            nc.vector.memset(zero_sb[:], 0.0)

            wcol_all = wres.tile([128, NB], DT.float32, tag="wcol")
            cnt_acc = wres.tile([128, E], DT.float32, tag="cnta")
            pbar_acc = wres.tile([128, E], DT.float32, tag="pbara")
            spz_sb = wres.tile([128, NB], DT.float32, tag="spz")
            cap_acc = wres.tile([128, 1], DT.float32, tag="capacc")
            nc.vector.memset(cnt_acc[:], 0.0)
            nc.vector.memset(pbar_acc[:], 0.0)
            nc.vector.memset(cap_acc[:], 0.0)

            if RB is None:
                partial = dram.tile([T, H], w_dtype)
                rs_out = dram.tile([T // N_CORES, H], w_dtype)
                for r in range(0, T, 128):
                    nc.sync.dma_start(out=partial[r:r + 128, :], in_=zero_sb[:])
            else:
                partial_lo = dram.tile([RB, H], w_dtype)
                partial_hi = dram.tile([T - RB, H], w_dtype)
                rs_lo = dram.tile([RB // N_CORES, H], w_dtype)
                rs_hi = dram.tile([(T - RB) // N_CORES, H], w_dtype)
                for r in range(0, RB, 128):
                    n = min(128, RB - r)
                    nc.sync.dma_start(out=partial_lo[r:r + n, :], in_=zero_sb[0:n, :])
                for r in range(0, T - RB, 128):
                    n = min(128, T - RB - r)
                    nc.sync.dma_start(out=partial_hi[r:r + n, :], in_=zero_sb[0:n, :])

            # ---- router on all token blocks (token-major: tokens on partitions) ----
            for b in range(NB):
                ti = next(j for j, (c0_, sz_) in enumerate(tiles) if c0_ <= b * 128 < c0_ + sz_)
                loc = b * 128 - tiles[ti][0]
                psr = ps.tile([128, 40], DT.float32, tag="psg")
                for h in range(NH):
                    nc.tensor.matmul(psr[:], x32_sb[h][ti][:, loc:loc + 128], rc_sb[h][:],
                                     start=(h == 0), stop=(h == NH - 1))
                L = rt.tile([128, 40], DT.float32, tag="r")
                nc.scalar.activation(L[:], psr[:], AF.Copy)
                E8 = rt.tile([128, E], DT.float32, tag="r")
                nc.scalar.activation(E8[:], L[:, 0:E], AF.Exp)
                Ssum = rt.tile([128, 1], DT.float32, tag="rs")
                nc.vector.tensor_reduce(Ssum[:], E8[:], AXIS_X, ALU.add)
                R = rt.tile([128, 1], DT.float32, tag="rs")
                nc.vector.reciprocal(R[:], Ssum[:])
                P = rt.tile([128, E], DT.float32, tag="r")
                nc.vector.tensor_scalar_mul(P[:], E8[:], R[:])
                M1 = rt.tile([128, 1], DT.float32, tag="rs")
                nc.vector.tensor_reduce(M1[:], P[:], AXIS_X, ALU.max)
                msk1 = rt.tile([128, E], DT.float32, tag="r")
                nc.vector.tensor_scalar(msk1[:], P[:], M1[:], None, ALU.is_ge)
                Pm = rt.tile([128, E], DT.float32, tag="r")
                nc.vector.scalar_tensor_tensor(Pm[:], msk1[:], -1e30, P[:], ALU.mult, ALU.add)
                M2 = rt.tile([128, 1], DT.float32, tag="rs")
                nc.vector.tensor_reduce(M2[:], Pm[:], AXIS_X, ALU.max)
                msk2 = rt.tile([128, E], DT.float32, tag="r")
                nc.vector.tensor_scalar(msk2[:], P[:], M2[:], None, ALU.is_equal)
                e1 = rt.tile([128, 1], DT.float32, tag="rs")
                nc.scalar.activation(e1[:], M1[:], AF.Exp)
                e2 = rt.tile([128, 1], DT.float32, tag="rs")
                nc.scalar.activation(e2[:], M2[:], AF.Exp)
                den = rt.tile([128, 1], DT.float32, tag="rs")
                nc.vector.tensor_tensor(den[:], e1[:], e2[:], ALU.add)
                rd = rt.tile([128, 1], DT.float32, tag="rs")
                nc.vector.reciprocal(rd[:], den[:])
                # dyn_cap via tanh (keeps ACT on one table set):
                # sigmoid(z) = 0.5*tanh(0.5 z) + 0.5 ; capb128 is pre-scaled by 0.5
                dc = rt.tile([128, 1], DT.float32, tag="rs")
                nc.scalar.activation(dc[:], L[:, 32:33], AF.Tanh, bias=capb_sb[:], scale=0.5)
                nc.vector.tensor_scalar(dc[:], dc[:], 0.5, 0.5, ALU.mult, ALU.add)
                s1 = rt.tile([128, 1], DT.float32, tag="rs")
                nc.vector.tensor_tensor(s1[:], e1[:], rd[:], ALU.mult)
                nc.vector.tensor_tensor(s1[:], s1[:], dc[:], ALU.mult)
                s2 = rt.tile([128, 1], DT.float32, tag="rs")
                nc.vector.tensor_tensor(s2[:], e2[:], rd[:], ALU.mult)
                nc.vector.tensor_tensor(s2[:], s2[:], dc[:], ALU.mult)
                w8 = rt.tile([128, E], DT.float32, tag="r")
                nc.vector.tensor_scalar_mul(w8[:], msk1[:], s1[:])
                tmp = rt.tile([128, E], DT.float32, tag="r")
                nc.vector.tensor_scalar_mul(tmp[:], msk2[:], s2[:])
                nc.vector.tensor_tensor(w8[:], w8[:], tmp[:], ALU.add)
                nc.vector.tensor_tensor(w8[:], w8[:], wsel_sb[:], ALU.mult)
                nc.vector.tensor_reduce(wcol_all[:, b:b + 1], w8[:], AXIS_X, ALU.add)

                vmb = vm_sb[:, b:b + 1]
                m12 = rt.tile([128, E], DT.float32, tag="r")
                nc.vector.tensor_tensor(m12[:], msk1[:], msk2[:], ALU.add)
                nc.vector.tensor_scalar_mul(m12[:], m12[:], vmb)
                nc.vector.tensor_tensor(cnt_acc[:], cnt_acc[:], m12[:], ALU.add)
                Pv = rt.tile([128, E], DT.float32, tag="r")
                nc.vector.tensor_scalar_mul(Pv[:], P[:], vmb)
                nc.vector.tensor_tensor(pbar_acc[:], pbar_acc[:], Pv[:], ALU.add)
                EP = rt.tile([128, E], DT.float32, tag="r")
                nc.scalar.activation(EP[:], P[:], AF.Exp)
                nc.vector.tensor_reduce(spz_sb[:, b:b + 1], EP[:], AXIS_X, ALU.add)
                dcv = rt.tile([128, 1], DT.float32, tag="rs")
                nc.vector.tensor_tensor(dcv[:], dc[:], vmb, ALU.mult)
                nc.vector.tensor_tensor(cap_acc[:], cap_acc[:], dcv[:], ALU.add)

            def emit_aux():
                # ---- aux partial sums -> one row via ones-matmul ----
                psx = ps.tile([1, 24], DT.float32, tag="psg")
                aux_cat = wres.tile([128, 24], DT.float32, tag="auxcat")
                nc.vector.memset(aux_cat[:], 0.0)
                nc.vector.tensor_copy(aux_cat[:, 0:E], cnt_acc[:])
                nc.vector.tensor_copy(aux_cat[:, E:2 * E], pbar_acc[:])
                nc.vector.tensor_copy(aux_cat[:, 2 * E:2 * E + 1], cap_acc[:])
                nc.tensor.matmul(psx[:], ones_sb[:], aux_cat[:], start=True, stop=True)
                aux_sb = wres.tile([1, 24], DT.float32, tag="auxsb")
                nc.vector.tensor_copy(aux_sb[:], psx[:])
                nc.sync.dma_start(out=aux_o[:], in_=aux_sb[:])
                nc.sync.dma_start(out=spz_o[:], in_=spz_sb[:])

            # ---- SwiGLU FFN on the gathered token set ----
            for ti, (c0, sz) in enumerate(tiles):
                hid = []
                for i in range(NI):
                    q, iloc = i * NQ // NI, (i % (NI // NQ)) * 128
                    psg = ps.tile([128, 512], DT.float32, tag="psg")
                    for h in range(NH):
                        nc.tensor.matmul(psg[:, 0:sz], gw_sb[h][q][:, iloc:iloc + 128],
                                         x16_sb[h][ti][:],
                                         start=(h == 0), stop=(h == NH - 1))
                    psu = ps.tile([128, 512], DT.float32, tag="psu")
                    for h in range(NH):
                        nc.tensor.matmul(psu[:, 0:sz], uw_sb[h][q][:, iloc:iloc + 128],
                                         x16_sb[h][ti][:],
                                         start=(h == 0), stop=(h == NH - 1))
                    # silu(g)*u with ACT on one table set: sigmoid(g)=0.5*tanh(g/2)+0.5
                    # so silu(g)*u = 0.5*g*((tanh(g/2)+1)*u)
                    th = epi.tile([128, 512], DT.float32, tag="th")
                    nc.scalar.activation(th[:, 0:sz], psg[:, 0:sz], AF.Tanh, scale=0.5)
                    sg = epi.tile([128, 512], DT.float32, tag="sg")
                    nc.vector.scalar_tensor_tensor(sg[:, 0:sz], th[:, 0:sz], 1.0, psu[:, 0:sz], ALU.add, ALU.mult)
                    ht = hidp.tile([128, 512], w_dtype, tag="hid")
                    nc.vector.scalar_tensor_tensor(ht[:, 0:sz], sg[:, 0:sz], 0.5, psg[:, 0:sz], ALU.mult, ALU.mult)
                    hid.append(ht)
                for tb in range(sz // 128):
                    b = (c0 + tb * 128) // 128
                    t0 = tb * 128
                    psyA = ps.tile([128, 384], DT.float32, tag="psyA")
                    psyB = ps.tile([128, 384], DT.float32, tag="psyB")
                    for i in range(NI):
                        nc.tensor.matmul(psyA[:], hid[i][:, t0:t0 + 128], dw_sb[i][:, 0:384],
                                         start=(i == 0), stop=(i == NI - 1))
                    for i in range(NI):
                        nc.tensor.matmul(psyB[:], hid[i][:, t0:t0 + 128], dw_sb[i][:, 384:768],
                                         start=(i == 0), stop=(i == NI - 1))
                    y_sb = epi.tile([128, H], w_dtype, tag="ysb")
                    nc.vector.tensor_scalar_mul(y_sb[:, 0:384], psyA[:], wcol_all[:, b:b + 1])
                    nc.vector.tensor_scalar_mul(y_sb[:, 384:768], psyB[:], wcol_all[:, b:b + 1])
                    if RB is None:
                        starget, sbound = partial, T - 1
                    elif b < 5:
                        starget, sbound = partial_lo, RB - 1
                    else:
                        starget, sbound = partial_hi, T - RB - 1
                    nc.gpsimd.indirect_dma_start(
                        out=starget[:],
                        out_offset=bass.IndirectOffsetOnAxis(ap=idx_sb[:, b:b + 1], axis=0),
                        in_=y_sb[:],
                        in_offset=None,
                        bounds_check=sbound,
                        oob_is_err=False,
                    )
                if ti == 0:
                    emit_aux()

            # ---- combine expert contributions across the 8 cores ----
            if RB is None:
                nc.gpsimd.collective_compute(
                    "ReduceScatter", ALU.add,
                    replica_groups=[list(range(N_CORES))],
                    ins=[partial.opt()], outs=[rs_out.opt()],
                )
                nc.sync.dma_start(out=out_shard[:], in_=rs_out[:])
            else:
                nc.gpsimd.collective_compute(
                    "ReduceScatter", ALU.add,
                    replica_groups=[list(range(N_CORES))],
                    ins=[partial_lo.opt()], outs=[rs_lo.opt()],
                )
                nc.sync.dma_start(out=out_lo[:], in_=rs_lo[:])
                nc.gpsimd.collective_compute(
                    "ReduceScatter", ALU.add,
                    replica_groups=[list(range(N_CORES))],
                    ins=[partial_hi.opt()], outs=[rs_hi.opt()],
                )
                nc.sync.dma_start(out=out_hi[:], in_=rs_hi[:])

    nc.compile()
    return nc


_NC_CACHE = {}


def _get_nc(C, R=None):
    if (C, R) not in _NC_CACHE:
        _NC_CACHE[(C, R)] = _build(C, R)
    return _NC_CACHE[(C, R)]


def _find_split(tok_lists):
    # boundary R: lo-tokens (<R) fit 5 blocks (640), hi-tokens fit 4 (512)
    for R in range(2048, 3072, 8):
        if all(int((t < R).sum()) <= 616 and int((t >= R).sum()) <= 500 for t in tok_lists):
            return R
    return None


def kernel(x, router_w, cap_w, cap_b, gate_w, up_w, down_w):
    x = np.asarray(x, np.float32)
    router_w = np.asarray(router_w, np.float32)
    cap_w = np.asarray(cap_w, np.float32)
    cap_b = np.asarray(cap_b, np.float32)
    gate_w = np.asarray(gate_w, np.float32)
    up_w = np.asarray(up_w, np.float32)
    down_w = np.asarray(down_w, np.float32)
    w_np = np.float16

    xf = x.reshape(T, H)
    xT = np.ascontiguousarray(xf.T)                      # [H, T]
    rc = np.zeros((40, H), np.float32)
    rc[:E] = router_w
    rc[32] = cap_w[0]
    rcT = np.ascontiguousarray(rc.T)                     # [H, 40]
    capb128 = np.broadcast_to(0.5 * cap_b.reshape(1, 1), (128, 1)).astype(np.float32).copy()

    # host top-2 routing (only to build the expert-parallel gather lists)
    logits = xf @ router_w.T                             # [T, E]
    i1 = np.argmax(logits, axis=1)
    l2 = logits.copy()
    l2[np.arange(T), i1] = -np.inf
    i2 = np.argmax(l2, axis=1)

    tok_lists = [np.where((i1 == c) | (i2 == c))[0] for c in range(N_CORES)]
    max_cnt = max(len(t) for t in tok_lists)
    C = max(1152, -(-max_cnt // 128) * 128)              # capacity (block-multiple)
    R = _find_split(tok_lists) if C == 1152 else None
    NB = C // 128
    nc = _get_nc(C, R)

    in_maps = []
    vmcols = []
    for c in range(N_CORES):
        toks = tok_lists[c]
        idx_arr = np.full(C, PAD_IDX, np.int64)
        if R is None:
            n = len(toks)
            idx_arr[:n] = toks
        else:
            lo, hi = toks[toks < R], toks[toks >= R]
            idx_arr[:len(lo)] = lo
            idx_arr[640:640 + len(hi)] = hi - R
        n = len(toks)
        vm_arr = (idx_arr != PAD_IDX).astype(np.float32)
        if R is None:
            gcols = np.where(idx_arr < T, idx_arr, 0)
        else:
            gcols = np.where(idx_arr != PAD_IDX, idx_arr, 0)
            gcols[640:] = np.where(idx_arr[640:] != PAD_IDX, idx_arr[640:] + R, 0)
        xg32 = np.ascontiguousarray(xT[:, gcols])
        wsel = np.zeros((128, E), np.float32)
        wsel[:, c] = 1.0
        vmcol = np.ascontiguousarray(vm_arr.reshape(NB, 128).T)
        vmcols.append(vmcol)
        in_maps.append({
            "xg32": xg32,
            "xg16": xg32.astype(w_np),
            "rcT": rcT,
            "capb128": capb128,
            "wsel": wsel,
            "idxcol": np.ascontiguousarray(idx_arr.reshape(NB, 128).T.astype(np.int32)),
            "vmcol": vmcol,
            "gwT": np.ascontiguousarray(gate_w[c].T).astype(w_np),
            "uwT": np.ascontiguousarray(up_w[c].T).astype(w_np),
            "dwT": np.ascontiguousarray(down_w[c].T).astype(w_np),
        })

    res = run_bass_kernel_spmd(nc, in_maps, list(range(N_CORES)))
    results = res.results

    if R is None:
        y = np.concatenate([results[c]["out_shard"] for c in range(N_CORES)], axis=0)
    else:
        y_lo = np.concatenate([results[c]["out_lo"] for c in range(N_CORES)], axis=0)
        y_hi = np.concatenate([results[c]["out_hi"] for c in range(N_CORES)], axis=0)
        y = np.concatenate([y_lo, y_hi], axis=0)
    out = y.astype(np.float32).reshape(B, S, H)

    aux = np.stack([results[c]["aux"][0] for c in range(N_CORES)])       # [8, 24]
    counts = np.array([aux[c, c] for c in range(N_CORES)], np.float64)
    pbar = aux[:, E:2 * E].sum(axis=0) / 2.0 / T
    capm = aux[:, 2 * E].sum() / 2.0 / T
    zsum = 0.0
    for c in range(N_CORES):
        spz = results[c]["spz"].astype(np.float64)
        zsum += float((np.log(np.where(spz > 0, spz, 1.0)) * vmcols[c]).sum())
    zsum = zsum / 2.0 / T

    lbl = np.float32(counts.var(ddof=1) / (T * K / E) ** 2)
    rzl = np.float32(zsum)
    dl = np.float32(-(pbar * np.log(pbar + 1e-8)).sum())
    cl = np.float32((capm - 0.6) ** 2)
    return (out, lbl, rzl, dl, cl)


# revision 22
# speedup vs baseline: 1.1552x; 1.0110x over previous
"""Trainium2 Bass kernel for nn_MixtureOfExperts_57045755625494.

Expert-parallel across 8 NeuronCores: core c owns expert e=c. The host
computes top-2 routing (fp32) only to build per-expert token gather lists;
all reference math runs on device:
  - fp32 router (PE matmul + exp/tanh on ACT) -> top-2 combine weights
  - dense SwiGLU FFN on the gathered token set (fp16 matmuls, fp32 PSUM)
  - per-token scatter back to a token-major partial buffer (indirect DMA)
  - ReduceScatter across the 8 cores combines expert contributions
  - aux-loss partial sums (counts / prob sums / exp-sums / cap sums)
Host gathers shards and finishes the four scalar losses from the
device-computed partials.

kernel(**inputs) takes the full unsharded inputs and returns
(out[2,2048,768] fp32, load_balance_loss, router_z_loss, diversity_loss,
capacity_loss) matching reference().
"""
import numpy as np

import concourse.bass as bass
import concourse.bacc as bacc
import concourse.mybir as mybir
import concourse.tile as tile
import bass_rust
from concourse.bass_utils import run_bass_kernel_spmd

AF = bass_rust.ActivationFunctionType
ALU = mybir.AluOpType
DT = mybir.dt

B, S, H, I, E, K = 2, 2048, 768, 3072, 8, 2
T = B * S
N_CORES = 8
NH = H // 128
NI = I // 128
PAD_IDX = 1 << 20

try:
    AXIS_X = mybir.AxisListType.X
except AttributeError:
    AXIS_X = bass_rust.AxisListType.X


def _build(C, RB=None, w_dtype=DT.float16):
    # R: row boundary for the split two-RS variant (blocks 0-4 scatter rows <R,
    # blocks 5.. scatter rows >=R), enabling the first ReduceScatter to overlap
    # the remaining FFN. R=None keeps the single-RS path.
    NB = C // 128
    # even-ish token tiles (multiples of 128, each <=512) to avoid a tiny
    # LDW-bound tail tile
    n_t = -(-C // 512)
    per = [C // n_t // 128] * n_t
    for j in range((C - sum(per) * 128) // 128):
        per[j] += 1
    tiles = []
    off = 0
    for blocks in per:
        tiles.append((off, blocks * 128))
        off += blocks * 128

    nc = bacc.Bacc("TRN2", target_bir_lowering=False, debug=False, num_devices=N_CORES)

    xg32 = nc.dram_tensor("xg32", [H, C], DT.float32, kind="ExternalInput").ap()
    xg16 = nc.dram_tensor("xg16", [H, C], w_dtype, kind="ExternalInput").ap()
    rcT = nc.dram_tensor("rcT", [H, 40], DT.float32, kind="ExternalInput").ap()
    capb = nc.dram_tensor("capb128", [128, 1], DT.float32, kind="ExternalInput").ap()
    wsel = nc.dram_tensor("wsel", [128, E], DT.float32, kind="ExternalInput").ap()
    idxc = nc.dram_tensor("idxcol", [128, NB], DT.int32, kind="ExternalInput").ap()
    vmc = nc.dram_tensor("vmcol", [128, NB], DT.float32, kind="ExternalInput").ap()
    gwT = nc.dram_tensor("gwT", [H, I], w_dtype, kind="ExternalInput").ap()
    uwT = nc.dram_tensor("uwT", [H, I], w_dtype, kind="ExternalInput").ap()
    dwT = nc.dram_tensor("dwT", [I, H], w_dtype, kind="ExternalInput").ap()

    if RB is None:
        out_shard = nc.dram_tensor("out_shard", [T // N_CORES, H], w_dtype, kind="ExternalOutput").ap()
    else:
        out_lo = nc.dram_tensor("out_lo", [RB // N_CORES, H], w_dtype, kind="ExternalOutput").ap()
        out_hi = nc.dram_tensor("out_hi", [(T - RB) // N_CORES, H], w_dtype, kind="ExternalOutput").ap()
    aux_o = nc.dram_tensor("aux", [1, 24], DT.float32, kind="ExternalOutput").ap()
    spz_o = nc.dram_tensor("spz", [128, NB], DT.float32, kind="ExternalOutput").ap()

    with tile.TileContext(nc) as tc:
        with (
            tc.tile_pool(name="wres", bufs=1) as wres,
            tc.tile_pool(name="rt", bufs=8) as rt,
            tc.tile_pool(name="hidp", bufs=NI + 6) as hidp,
            tc.tile_pool(name="epi", bufs=3) as epi,
            tc.tile_pool(name="ps", bufs=2, space="PSUM") as ps,
            tc.tile_pool(name="dram", bufs=1, space="DRAM") as dram,
        ):
            NQ = 4
            IQ = I // NQ
            gw_sb, uw_sb, dw_sb, rc_sb, x32_sb, x16_sb = [], [], [], [], [], []
            capb_sb = wres.tile([128, 1], DT.float32, tag="capb")
            nc.sync.dma_start(out=capb_sb[:], in_=capb[:])
            wsel_sb = wres.tile([128, E], DT.float32, tag="wsel")
            nc.sync.dma_start(out=wsel_sb[:], in_=wsel[:])
            idx_sb = wres.tile([128, NB], DT.int32, tag="idx")
            nc.sync.dma_start(out=idx_sb[:], in_=idxc[:])
            vm_sb = wres.tile([128, NB], DT.float32, tag="vm")
            nc.sync.dma_start(out=vm_sb[:], in_=vmc[:])
            for h in range(NH):
                t_ = wres.tile([128, 40], DT.float32, tag=f"rc{h}")
                nc.sync.dma_start(out=t_[:], in_=rcT[h * 128:(h + 1) * 128, :])
                rc_sb.append(t_)
            # x fp32 split by token tile: router blocks depend only on their chunk
            for h in range(NH):
                row = []
                for ti, (c0, sz) in enumerate(tiles):
                    t_ = wres.tile([128, sz], DT.float32, tag=f"x32{h}_{ti}", name=f"x32_{h}_{ti}")
                    nc.sync.dma_start(out=t_[:], in_=xg32[h * 128:(h + 1) * 128, c0:c0 + sz])
                    row.append(t_)
                x32_sb.append(row)
            # gate/up weights split into NQ column groups: GEMM1 i-chunk j needs only group j*NQ//NI
            for h in range(NH):
                gw_sb.append([None] * NQ)
                uw_sb.append([None] * NQ)
            for q in range(NQ):
                for h in range(NH):
                    t_ = wres.tile([128, IQ], w_dtype, tag=f"gw{h}_{q}", name=f"gw_{h}_{q}")
                    nc.sync.dma_start(out=t_[:], in_=gwT[h * 128:(h + 1) * 128, q * IQ:(q + 1) * IQ])
                    gw_sb[h][q] = t_
                for h in range(NH):
                    t_ = wres.tile([128, IQ], w_dtype, tag=f"uw{h}_{q}", name=f"uw_{h}_{q}")
                    nc.sync.dma_start(out=t_[:], in_=uwT[h * 128:(h + 1) * 128, q * IQ:(q + 1) * IQ])
                    uw_sb[h][q] = t_
                if q == 0:
                    for h in range(NH):
                        row = []
                        for ti, (c0, sz) in enumerate(tiles):
                            t_ = wres.tile([128, sz], w_dtype, tag=f"x16{h}_{ti}", name=f"x16_{h}_{ti}")
                            nc.sync.dma_start(out=t_[:], in_=xg16[h * 128:(h + 1) * 128, c0:c0 + sz])
                            row.append(t_)
                        x16_sb.append(row)
            for i in range(NI):
                t_ = wres.tile([128, H], w_dtype, tag=f"dw{i}")
                nc.sync.dma_start(out=t_[:], in_=dwT[i * 128:(i + 1) * 128, :])
                dw_sb.append(t_)

            ones_sb = wres.tile([128, 1], DT.float32, tag="ones")
            nc.vector.memset(ones_sb[:], 1.0)
            zero_sb = wres.tile([128, H], w_dtype, tag="zero")
            nc.vector.memset(zero_sb[:], 0.0)

            wcol_all = wres.tile([128, NB], DT.float32, tag="wcol")
            cnt_acc = wres.tile([128, E], DT.float32, tag="cnta")
            pbar_acc = wres.tile([128, E], DT.float32, tag="pbara")
            spz_sb = wres.tile([128, NB], DT.float32, tag="spz")
            cap_acc = wres.tile([128, 1], DT.float32, tag="capacc")
            nc.vector.memset(cnt_acc[:], 0.0)
            nc.vector.memset(pbar_acc[:], 0.0)
            nc.vector.memset(cap_acc[:], 0.0)

            if RB is None:
                partial = dram.tile([T, H], w_dtype)
                rs_out = dram.tile([T // N_CORES, H], w_dtype)
                for r in range(0, T, 128):
                    nc.sync.dma_start(out=partial[r:r + 128, :], in_=zero_sb[:])
            else:
                partial_lo = dram.tile([RB, H], w_dtype)
                partial_hi = dram.tile([T - RB, H], w_dtype)
                rs_lo = dram.tile([RB // N_CORES, H], w_dtype)
                rs_hi = dram.tile([(T - RB) // N_CORES, H], w_dtype)
                for r in range(0, RB, 128):
                    n = min(128, RB - r)
                    nc.sync.dma_start(out=partial_lo[r:r + n, :], in_=zero_sb[0:n, :])
                for r in range(0, T - RB, 128):
                    n = min(128, T - RB - r)
                    nc.sync.dma_start(out=partial_hi[r:r + n, :], in_=zero_sb[0:n, :])

            # ---- router on all token blocks (token-major: tokens on partitions) ----
            for b in range(NB):
                ti = next(j for j, (c0_, sz_) in enumerate(tiles) if c0_ <= b * 128 < c0_ + sz_)
                loc = b * 128 - tiles[ti][0]
                psr = ps.tile([128, 40], DT.float32, tag="psg")
                for h in range(NH):
                    nc.tensor.matmul(psr[:], x32_sb[h][ti][:, loc:loc + 128], rc_sb[h][:],
                                     start=(h == 0), stop=(h == NH - 1))
                L = rt.tile([128, 40], DT.float32, tag="r")
                nc.scalar.activation(L[:], psr[:], AF.Copy)
                E8 = rt.tile([128, E], DT.float32, tag="r")
                nc.scalar.activation(E8[:], L[:, 0:E], AF.Exp)
                Ssum = rt.tile([128, 1], DT.float32, tag="rs")
                nc.vector.tensor_reduce(Ssum[:], E8[:], AXIS_X, ALU.add)
                R = rt.tile([128, 1], DT.float32, tag="rs")
                nc.vector.reciprocal(R[:], Ssum[:])
                P = rt.tile([128, E], DT.float32, tag="r")
                nc.vector.tensor_scalar_mul(P[:], E8[:], R[:])
                M1 = rt.tile([128, 1], DT.float32, tag="rs")
                nc.vector.tensor_reduce(M1[:], P[:], AXIS_X, ALU.max)
                msk1 = rt.tile([128, E], DT.float32, tag="r")
                nc.vector.tensor_scalar(msk1[:], P[:], M1[:], None, ALU.is_ge)
                Pm = rt.tile([128, E], DT.float32, tag="r")
                nc.vector.scalar_tensor_tensor(Pm[:], msk1[:], -1e30, P[:], ALU.mult, ALU.add)
                M2 = rt.tile([128, 1], DT.float32, tag="rs")
                nc.vector.tensor_reduce(M2[:], Pm[:], AXIS_X, ALU.max)
                msk2 = rt.tile([128, E], DT.float32, tag="r")
                nc.vector.tensor_scalar(msk2[:], P[:], M2[:], None, ALU.is_equal)
                e1 = rt.tile([128, 1], DT.float32, tag="rs")
                nc.scalar.activation(e1[:], M1[:], AF.Exp)
                e2 = rt.tile([128, 1], DT.float32, tag="rs")
                nc.scalar.activation(e2[:], M2[:], AF.Exp)
                den = rt.tile([128, 1], DT.float32, tag="rs")
                nc.vector.tensor_tensor(den[:], e1[:], e2[:], ALU.add)
                rd = rt.tile([128, 1], DT.float32, tag="rs")
                nc.vector.reciprocal(rd[:], den[:])
                # dyn_cap via tanh (keeps ACT on one table set):
                # sigmoid(z) = 0.5*tanh(0.5 z) + 0.5 ; capb128 is pre-scaled by 0.5
                dc = rt.tile([128, 1], DT.float32, tag="rs")
                nc.scalar.activation(dc[:], L[:, 32:33], AF.Tanh, bias=capb_sb[:], scale=0.5)
                nc.vector.tensor_scalar(dc[:], dc[:], 0.5, 0.5, ALU.mult, ALU.add)
                s1 = rt.tile([128, 1], DT.float32, tag="rs")
                nc.vector.tensor_tensor(s1[:], e1[:], rd[:], ALU.mult)
                nc.vector.tensor_tensor(s1[:], s1[:], dc[:], ALU.mult)
                s2 = rt.tile([128, 1], DT.float32, tag="rs")
                nc.vector.tensor_tensor(s2[:], e2[:], rd[:], ALU.mult)
                nc.vector.tensor_tensor(s2[:], s2[:], dc[:], ALU.mult)
                w8 = rt.tile([128, E], DT.float32, tag="r")
                nc.vector.tensor_scalar_mul(w8[:], msk1[:], s1[:])
                tmp = rt.tile([128, E], DT.float32, tag="r")
                nc.vector.tensor_scalar_mul(tmp[:], msk2[:], s2[:])
                nc.vector.tensor_tensor(w8[:], w8[:], tmp[:], ALU.add)
                nc.vector.tensor_tensor(w8[:], w8[:], wsel_sb[:], ALU.mult)
                nc.vector.tensor_reduce(wcol_all[:, b:b + 1], w8[:], AXIS_X, ALU.add)

                vmb = vm_sb[:, b:b + 1]
                m12 = rt.tile([128, E], DT.float32, tag="r")
                nc.vector.tensor_tensor(m12[:], msk1[:], msk2[:], ALU.add)
                nc.vector.tensor_scalar_mul(m12[:], m12[:], vmb)
                nc.vector.tensor_tensor(cnt_acc[:], cnt_acc[:], m12[:], ALU.add)
                Pv = rt.tile([128, E], DT.float32, tag="r")
                nc.vector.tensor_scalar_mul(Pv[:], P[:], vmb)
                nc.vector.tensor_tensor(pbar_acc[:], pbar_acc[:], Pv[:], ALU.add)
                EP = rt.tile([128, E], DT.float32, tag="r")
                nc.scalar.activation(EP[:], P[:], AF.Exp)
                nc.vector.tensor_reduce(spz_sb[:, b:b + 1], EP[:], AXIS_X, ALU.add)
                dcv = rt.tile([128, 1], DT.float32, tag="rs")
                nc.vector.tensor_tensor(dcv[:], dc[:], vmb, ALU.mult)
                nc.vector.tensor_tensor(cap_acc[:], cap_acc[:], dcv[:], ALU.add)

            def emit_aux():
                # ---- aux partial sums -> one row via ones-matmul ----
                psx = ps.tile([1, 24], DT.float32, tag="psg")
                aux_cat = wres.tile([128, 24], DT.float32, tag="auxcat")
                nc.vector.memset(aux_cat[:], 0.0)
                nc.vector.tensor_copy(aux_cat[:, 0:E], cnt_acc[:])
                nc.vector.tensor_copy(aux_cat[:, E:2 * E], pbar_acc[:])
                nc.vector.tensor_copy(aux_cat[:, 2 * E:2 * E + 1], cap_acc[:])
                nc.tensor.matmul(psx[:], ones_sb[:], aux_cat[:], start=True, stop=True)
                aux_sb = wres.tile([1, 24], DT.float32, tag="auxsb")
                nc.vector.tensor_copy(aux_sb[:], psx[:])
                nc.sync.dma_start(out=aux_o[:], in_=aux_sb[:])
                nc.sync.dma_start(out=spz_o[:], in_=spz_sb[:])

            # ---- SwiGLU FFN on the gathered token set ----
            for ti, (c0, sz) in enumerate(tiles):
                hid = []
                for i in range(NI):
                    q, iloc = i * NQ // NI, (i % (NI // NQ)) * 128
                    psg = ps.tile([128, 512], DT.float32, tag="psg")
                    for h in range(NH):
                        nc.tensor.matmul(psg[:, 0:sz], gw_sb[h][q][:, iloc:iloc + 128],
                                         x16_sb[h][ti][:],
                                         start=(h == 0), stop=(h == NH - 1))
                    psu = ps.tile([128, 512], DT.float32, tag="psu")
                    for h in range(NH):
                        nc.tensor.matmul(psu[:, 0:sz], uw_sb[h][q][:, iloc:iloc + 128],
                                         x16_sb[h][ti][:],
                                         start=(h == 0), stop=(h == NH - 1))
                    # silu(g)*u with ACT on one table set: sigmoid(g)=0.5*tanh(g/2)+0.5
                    # so silu(g)*u = 0.5*g*((tanh(g/2)+1)*u)
                    th = epi.tile([128, 512], DT.float32, tag="th")
                    nc.scalar.activation(th[:, 0:sz], psg[:, 0:sz], AF.Tanh, scale=0.5)
                    sg = epi.tile([128, 512], DT.float32, tag="sg")
                    nc.vector.scalar_tensor_tensor(sg[:, 0:sz], th[:, 0:sz], 1.0, psu[:, 0:sz], ALU.add, ALU.mult)
                    ht = hidp.tile([128, 512], w_dtype, tag="hid")
                    nc.vector.scalar_tensor_tensor(ht[:, 0:sz], sg[:, 0:sz], 0.5, psg[:, 0:sz], ALU.mult, ALU.mult)
                    hid.append(ht)
                for tb in range(sz // 128):
                    b = (c0 + tb * 128) // 128
                    t0 = tb * 128
                    psyA = ps.tile([128, 384], DT.float32, tag="psyA")
                    psyB = ps.tile([128, 384], DT.float32, tag="psyB")
                    for i in range(NI):
                        nc.tensor.matmul(psyA[:], hid[i][:, t0:t0 + 128], dw_sb[i][:, 0:384],
                                         start=(i == 0), stop=(i == NI - 1))
                    for i in range(NI):
                        nc.tensor.matmul(psyB[:], hid[i][:, t0:t0 + 128], dw_sb[i][:, 384:768],
                                         start=(i == 0), stop=(i == NI - 1))
                    y_sb = epi.tile([128, H], w_dtype, tag="ysb")
                    nc.vector.tensor_scalar_mul(y_sb[:, 0:384], psyA[:], wcol_all[:, b:b + 1])
                    nc.vector.tensor_scalar_mul(y_sb[:, 384:768], psyB[:], wcol_all[:, b:b + 1])
                    if RB is None:
                        starget, sbound = partial, T - 1
                    elif b < 5:
                        starget, sbound = partial_lo, RB - 1
                    else:
                        starget, sbound = partial_hi, T - RB - 1
                    nc.gpsimd.indirect_dma_start(
                        out=starget[:],
                        out_offset=bass.IndirectOffsetOnAxis(ap=idx_sb[:, b:b + 1], axis=0),
                        in_=y_sb[:],
                        in_offset=None,
                        bounds_check=sbound,
                        oob_is_err=False,
                    )
                if ti == 0:
                    emit_aux()

            # ---- combine expert contributions across the 8 cores ----
            if RB is None:
                nc.gpsimd.collective_compute(
                    "ReduceScatter", ALU.add,
                    replica_groups=[list(range(N_CORES))],
                    ins=[partial.opt()], outs=[rs_out.opt()],
                )
                nc.sync.dma_start(out=out_shard[:], in_=rs_out[:])
            else:
                nc.gpsimd.collective_compute(
                    "ReduceScatter", ALU.add,
                    replica_groups=[list(range(N_CORES))],
                    ins=[partial_lo.opt()], outs=[rs_lo.opt()],
                )
                nc.sync.dma_start(out=out_lo[:], in_=rs_lo[:])
                nc.gpsimd.collective_compute(
                    "ReduceScatter", ALU.add,
                    replica_groups=[list(range(N_CORES))],
                    ins=[partial_hi.opt()], outs=[rs_hi.opt()],
                )
                nc.sync.dma_start(out=out_hi[:], in_=rs_hi[:])

    nc.compile()
    return nc


_NC_CACHE = {}


def _get_nc(C, R=None):
    if (C, R) not in _NC_CACHE:
        _NC_CACHE[(C, R)] = _build(C, R)
    return _NC_CACHE[(C, R)]


def _find_split(tok_lists):
    # boundary R: lo-tokens (<R) fit 5 blocks (640), hi-tokens fit 4 (512).
    # Prefer the LARGEST feasible R so the overlapped first ReduceScatter
    # covers as many output rows as possible and the tail RS shrinks.
    for R in range(3064, 2048, -8):
        if all(int((t < R).sum()) <= 616 and int((t >= R).sum()) <= 500 for t in tok_lists):
            return R
    return None


def kernel(x, router_w, cap_w, cap_b, gate_w, up_w, down_w):
    x = np.asarray(x, np.float32)
    router_w = np.asarray(router_w, np.float32)
    cap_w = np.asarray(cap_w, np.float32)
    cap_b = np.asarray(cap_b, np.float32)
    gate_w = np.asarray(gate_w, np.float32)
    up_w = np.asarray(up_w, np.float32)
    down_w = np.asarray(down_w, np.float32)
    w_np = np.float16

    xf = x.reshape(T, H)
    xT = np.ascontiguousarray(xf.T)                      # [H, T]
    rc = np.zeros((40, H), np.float32)
    rc[:E] = router_w
    rc[32] = cap_w[0]
    rcT = np.ascontiguousarray(rc.T)                     # [H, 40]
    capb128 = np.broadcast_to(0.5 * cap_b.reshape(1, 1), (128, 1)).astype(np.float32).copy()

    # host top-2 routing (only to build the expert-parallel gather lists)
    logits = xf @ router_w.T                             # [T, E]
    i1 = np.argmax(logits, axis=1)
    l2 = logits.copy()
    l2[np.arange(T), i1] = -np.inf
    i2 = np.argmax(l2, axis=1)

    tok_lists = [np.where((i1 == c) | (i2 == c))[0] for c in range(N_CORES)]
    max_cnt = max(len(t) for t in tok_lists)
    C = max(1152, -(-max_cnt // 128) * 128)              # capacity (block-multiple)
    R = _find_split(tok_lists) if C == 1152 else None
    NB = C // 128
    nc = _get_nc(C, R)

    in_maps = []
    vmcols = []
    for c in range(N_CORES):
        toks = tok_lists[c]
        idx_arr = np.full(C, PAD_IDX, np.int64)
        if R is None:
            n = len(toks)
            idx_arr[:n] = toks
        else:
            lo, hi = toks[toks < R], toks[toks >= R]
            idx_arr[:len(lo)] = lo
            idx_arr[640:640 + len(hi)] = hi - R
        n = len(toks)
        vm_arr = (idx_arr != PAD_IDX).astype(np.float32)
        if R is None:
            gcols = np.where(idx_arr < T, idx_arr, 0)
        else:
            gcols = np.where(idx_arr != PAD_IDX, idx_arr, 0)
            gcols[640:] = np.where(idx_arr[640:] != PAD_IDX, idx_arr[640:] + R, 0)
        xg32 = np.ascontiguousarray(xT[:, gcols])
        wsel = np.zeros((128, E), np.float32)
        wsel[:, c] = 1.0
        vmcol = np.ascontiguousarray(vm_arr.reshape(NB, 128).T)
        vmcols.append(vmcol)
        in_maps.append({
            "xg32": xg32,
            "xg16": xg32.astype(w_np),
            "rcT": rcT,
            "capb128": capb128,
            "wsel": wsel,
            "idxcol": np.ascontiguousarray(idx_arr.reshape(NB, 128).T.astype(np.int32)),
            "vmcol": vmcol,
            "gwT": np.ascontiguousarray(gate_w[c].T).astype(w_np),
            "uwT": np.ascontiguousarray(up_w[c].T).astype(w_np),
            "dwT": np.ascontiguousarray(down_w[c].T).astype(w_np),
        })

    res = run_bass_kernel_spmd(nc, in_maps, list(range(N_CORES)))
    results = res.results

    if R is None:
        y = np.concatenate([results[c]["out_shard"] for c in range(N_CORES)], axis=0)
    else:
        y_lo = np.concatenate([results[c]["out_lo"] for c in range(N_CORES)], axis=0)
        y_hi = np.concatenate([results[c]["out_hi"] for c in range(N_CORES)], axis=0)
        y = np.concatenate([y_lo, y_hi], axis=0)
    out = y.astype(np.float32).reshape(B, S, H)

    aux = np.stack([results[c]["aux"][0] for c in range(N_CORES)])       # [8, 24]
    counts = np.array([aux[c, c] for c in range(N_CORES)], np.float64)
    pbar = aux[:, E:2 * E].sum(axis=0) / 2.0 / T
    capm = aux[:, 2 * E].sum() / 2.0 / T
    zsum = 0.0
    for c in range(N_CORES):
        spz = results[c]["spz"].astype(np.float64)
        zsum += float((np.log(np.where(spz > 0, spz, 1.0)) * vmcols[c]).sum())
    zsum = zsum / 2.0 / T

    lbl = np.float32(counts.var(ddof=1) / (T * K / E) ** 2)
    rzl = np.float32(zsum)
    dl = np.float32(-(pbar * np.log(pbar + 1e-8)).sum())
    cl = np.float32((capm - 0.6) ** 2)
    return (out, lbl, rzl, dl, cl)
